# revision 4
# baseline (speedup 1.0000x reference)
"""Channel-selective 1x1-conv MLP + per-pixel sorted top-k for TRN2 (8 cores).

Per pixel p: keys = A @ x[:,p] + ceff  (A = w2@w1, ceff = w2@b1+b2, both
precomputed on device once), xi = top-128 desc indices of keys,
out[k,p] = x[xi_k,p] * keys[xi_k,p]  (sigmoid is monotone -> never computed).

v3 design (vs the v2 max8/match_replace chain at ~19.9us/tile DVE):
 - Top-128 selection+sort via a 71-op BITONIC merge network
   (tensor_tensor min/max) BATCHED 10 column-tiles wide per instruction
   ([128, 2560] ops), which amortizes the ~84ns/op DVE overhead:
   ~9.1us/tile vs 19.9 for the chain.  Reversed-operand APs (negative
   stride) give uniform-direction merges; stage 8 computes only the top
   half.  Channel idx is embedded in the low 8 mantissa bits (fused
   scalar_tensor_tensor AND/OR); sort order exact except ~2^-16-relative
   key ties, fixed by a 2-layer odd-even cleanup on exact keys.
 - Pool (GpSimd) work batched per 10-tile group and minimized (it is the
   #2 engine at ~17ns/local_scatter index): 2 rank-inversion scatters
   (num_elems cap 2047 forces the split) + ONE payload gather (x as f16)
   + ONE exact-key low-plane gather.  The -1 rank adjust is avoided by
   scattering slot+1 ranks and gathering through a 1-slot pad (duplicate
   writes of unselected channels land in the pad).  The exact sorted key
   is rebuilt as (sorted item bits[31:8] | gathered low byte), needing no
   second plane gather or interleave.
 - Engine placement tuned by A/B bench: strided i16 plane ops are fast on
   DVE, very slow on Pool; plain f32 buffer copies go to ACT; the g32
   snapshot is a contiguous DVE copy (cross-engine WAR ping-pongs stall
   the DVE queue).
 - 4-deep group pipeline (stages at ring offsets 0,1,3,4) so Pool runs a
   full group behind DVE without convoying it.

Measured: rel L2 err 0.0059 (gate 2e-2), HW ~1.30ms vs 3.18ms for v2 on
the same bench (in-NEFF repeat-loop differencing, best of 3).
Bench note: consecutive device benches vary +-10-30%.
"""

import numpy as np

import concourse.bass as bass
import concourse.mybir as mybir
from concourse import bacc
from concourse.tile import TileContext
from concourse.masks import make_identity

B, C, H, W = 16, 256, 80, 80
MED, K = 128, 128
HWP = H * W
NCORES = 8
BPC = B // NCORES
P = 128
COLS = HWP // P          # 50
NT = BPC * COLS          # 100 tiles/core
BT = 10                  # tiles per sort batch (group); divides NT

F32 = mybir.dt.float32
F16 = mybir.dt.float16
I32 = mybir.dt.int32
I16 = mybir.dt.int16
U8 = mybir.dt.uint8
ACT_COPY = mybir.ActivationFunctionType.Copy
ALU = mybir.AluOpType


def bitonic_topk(nc, wk, kemb, nt):
    """Batched bitonic top-128-desc on nt side-by-side 256-col pixel tiles.
    kemb: [P, nt*256] f32 (distinct keys).  Returns [P, nt*128] tile."""
    Wd = nt * C
    cur = kemb
    for k in range(1, 8):
        m = 1 << (k - 1)
        nxt = wk.tile([P, Wd], F32, tag="bt", bufs=2)
        curB = cur.rearrange("p (n nb x) -> p n nb x", n=nt, x=2 * m)
        nxtB = nxt.rearrange("p (n nb x) -> p n nb x", n=nt, x=2 * m)
        A = curB[:, :, :, 0:m]
        Brev = curB[:, :, :, 2 * m - 1 : m - 1 : -1] if m > 1 else curB[:, :, :, 1:2]
        nc.vector.tensor_tensor(nxtB[:, :, :, 0:m], A, Brev, ALU.max)
        nc.vector.tensor_tensor(nxtB[:, :, :, m : 2 * m], A, Brev, ALU.min)
        cur = nxt
        s = m // 2
        while s >= 1:
            nxt = wk.tile([P, Wd], F32, tag="bt", bufs=2)
            V = cur.rearrange("p (nnb two s) -> p nnb two s", two=2, s=s)
            NV = nxt.rearrange("p (nnb two s) -> p nnb two s", two=2, s=s)
            nc.vector.tensor_tensor(NV[:, :, 0, :], V[:, :, 0, :], V[:, :, 1, :], ALU.max)
            nc.vector.tensor_tensor(NV[:, :, 1, :], V[:, :, 0, :], V[:, :, 1, :], ALU.min)
            cur = nxt
            s //= 2
    top = wk.tile([P, nt * K], F32, tag="btop", bufs=2)
    curB = cur.rearrange("p (n x) -> p n x", n=nt)
    topB = top.rearrange("p (n x) -> p n x", n=nt)
    nc.vector.tensor_tensor(topB, curB[:, :, 0:K], curB[:, :, C - 1 : K - 1 : -1], ALU.max)
    s = 64
    while s >= 1:
        nxt = wk.tile([P, nt * K], F32, tag="btop", bufs=2)
        V = top.rearrange("p (nnb two s) -> p nnb two s", two=2, s=s)
        NV = nxt.rearrange("p (nnb two s) -> p nnb two s", two=2, s=s)
        nc.vector.tensor_tensor(NV[:, :, 0, :], V[:, :, 0, :], V[:, :, 1, :], ALU.max)
        nc.vector.tensor_tensor(NV[:, :, 1, :], V[:, :, 0, :], V[:, :, 1, :], ALU.min)
        top = nxt
        s //= 2
    return top


def build(n_tiles: int = NT, repeat: int = 1, batch: int = BT,
          no_cleanup: bool = False, stage: str = "full") -> "bacc.Bacc":
    nc = bacc.Bacc(None, target_bir_lowering=False, debug=True)
    x_ext = nc.declare_dram_parameter("x", [BPC, C, HWP], F32, isOutput=False)
    w1_ext = nc.declare_dram_parameter("w1", [MED, C], F32, isOutput=False)
    b1_ext = nc.declare_dram_parameter("b1", [MED, 1], F32, isOutput=False)
    w2_ext = nc.declare_dram_parameter("w2", [C, MED], F32, isOutput=False)
    b2_ext = nc.declare_dram_parameter("b2", [1, C], F32, isOutput=False)
    out_ext = nc.declare_dram_parameter("out", [BPC, K, HWP], F32, isOutput=True)

    assert n_tiles % batch == 0, (n_tiles, batch)
    BTC = batch * C      # group key width
    BTK = batch * K      # group topk width

    with TileContext(nc) as tc:
        with (
            tc.tile_pool(name="const", bufs=1) as cpool,
            tc.tile_pool(name="io", bufs=4) as iop,
            tc.tile_pool(name="wk", bufs=2) as wk,
            tc.tile_pool(name="psum", bufs=2, space="PSUM") as pp,
        ):
            from concourse import library_config
            nc.gpsimd.load_library(library_config.local_scatter)

            # ---------------- constants ----------------
            ident = cpool.tile([P, P], F32)
            make_identity(nc, ident)
            ident16 = cpool.tile([P, P], F16)
            nc.vector.tensor_copy(ident16, ident)
            ones_row = cpool.tile([1, P], F32)
            nc.vector.memset(ones_row, 1.0)

            # iotap[p, i] = i+1 (f32) via lower-tri matmul
            ones_sq = cpool.tile([P, P], F32)
            nc.vector.memset(ones_sq, 1.0)
            tri = cpool.tile([P, P], F32)
            nc.gpsimd.affine_select(
                out=tri, in_=ones_sq, compare_op=ALU.is_ge, fill=0.0,
                base=0, pattern=[[1, P]], channel_multiplier=-1,
            )
            iotap = pp.tile([P, K], F32, tag="tr")
            nc.tensor.matmul(iotap, lhsT=ones_sq, rhs=tri, start=True, stop=True)

            # iota1B[p, i*K + j] = i*K + j + 1 (i16): rank-scatter data with
            # global group offsets
            iota1B = cpool.tile([P, BTK], I16)
            for i in range(batch):
                nc.scalar.activation(iota1B[:, i * K : (i + 1) * K], iotap,
                                     ACT_COPY, bias=float(i * K))
            # iotaB[p, i*C + c] = c (i32), repeated per group slot
            iotaB = cpool.tile([P, BTC], I32)
            for i in range(batch):
                nc.scalar.activation(iotaB[:, i * C : i * C + K], iotap,
                                     ACT_COPY, bias=-1.0)
                nc.scalar.activation(iotaB[:, i * C + K : (i + 1) * C], iotap,
                                     ACT_COPY, bias=127.0)
            # offsB[p, i*K + j] = (i % half)*256 (i32): LOCAL element offsets
            # for the split rank scatters
            half = batch // 2
            offsB = cpool.tile([P, BTK], I32)
            for i in range(batch):
                nc.vector.memset(offsB[:, i * K : (i + 1) * K], (i % half) * C)
            # per-partition int scalars for the fused bitwise ops (int imm
            # scalars lower as f32 and fail the verifier)
            sc_m256 = cpool.tile([P, 1], I32)
            nc.vector.memset(sc_m256, -256)
            sc_ff = cpool.tile([P, 1], I32)
            nc.vector.memset(sc_ff, 0xFF)
            sc_ff16 = cpool.tile([P, 1], I16)
            nc.vector.memset(sc_ff16, 0xFF)
            offsB16 = cpool.tile([P, BTK], I16)
            for i in range(batch):
                nc.vector.memset(offsB16[:, i * K : (i + 1) * K], (i % half) * C)

            # ---------------- weights ----------------
            # w2T [m, c]
            w2T = cpool.tile([MED, C], F32)
            for h in range(2):
                w2sb = iop.tile([P, MED], F32, tag="wload2")
                nc.sync.dma_start(out=w2sb, in_=w2_ext[P * h : P * (h + 1), :])
                tp = pp.tile([P, P], F32, tag="tr")
                nc.tensor.transpose(tp, w2sb, ident)
                nc.scalar.copy(w2T[:, P * h : P * (h + 1)], tp)

            # AT halves: AT[c', c] = sum_m w1[m, c'] w2[c, m]
            w1sb = iop.tile([MED, C], F32, tag="wload")
            nc.sync.dma_start(out=w1sb, in_=w1_ext[:, :])
            ATs = []
            for h in range(2):
                atp = pp.tile([P, C], F32, tag="keys", bufs=2)
                nc.tensor.matmul(atp, lhsT=w1sb[:, P * h : P * (h + 1)], rhs=w2T,
                                 start=True, stop=True)
                at = cpool.tile([P, C], F32, name=f"AT{h}")
                nc.scalar.copy(at, atp)
                ATs.append(at)

            # ceff[1, c] = w2 @ b1 + b2
            b1col = iop.tile([MED, 1], F32, tag="b1c")
            nc.sync.dma_start(out=b1col, in_=b1_ext[:, :])
            b2row = iop.tile([1, C], F32, tag="b2r")
            nc.sync.dma_start(out=b2row, in_=b2_ext[:, :])
            cep = pp.tile([1, C], F32, tag="ce", bufs=1)
            nc.tensor.matmul(cep, lhsT=b1col, rhs=w2T, start=True, stop=True)
            ceff = cpool.tile([1, C], F32)
            nc.vector.tensor_tensor(ceff, cep, b2row, ALU.add)

            # ---------------- main loop ----------------
            groups = [list(range(g, min(g + batch, n_tiles)))
                      for g in range(0, n_tiles, batch)]

            def stA(ts):
                """DMA + GEMM keys (per tile) into group buffers; payload
                transposes; then group embed + sort + idx extract (DVE)."""
                gb = len(ts)
                st = {"ts": ts}
                kbatch = wk.tile([P, BTC], F32, tag="kbatch", bufs=1)
                xT16 = wk.tile([P, BTC], F16, tag="xT16", bufs=3)
                for i, t in enumerate(ts):
                    b, j = divmod(t, COLS)
                    col = j * P
                    x0 = iop.tile([P, P], F32, tag="x0")
                    nc.sync.dma_start(out=x0, in_=x_ext[b, 0:P, col : col + P])
                    x1 = iop.tile([P, P], F32, tag="x1")
                    nc.sync.dma_start(out=x1, in_=x_ext[b, P:C, col : col + P])
                    kp = pp.tile([P, C], F32, tag="keys", bufs=2)
                    nc.tensor.matmul(kp, lhsT=x0, rhs=ATs[0], start=True, stop=False)
                    nc.tensor.matmul(kp, lhsT=x1, rhs=ATs[1], start=False, stop=False)
                    nc.tensor.matmul(kp, lhsT=ones_row, rhs=ceff, start=False, stop=True)
                    nc.scalar.copy(kbatch[:, i * C : (i + 1) * C], kp)
                    for h, xh in enumerate((x0, x1)):
                        tp = pp.tile([P, P], F32, tag="tr")
                        nc.tensor.transpose(tp, xh, ident)
                        nc.scalar.copy(xT16[:, i * C + h * P : i * C + (h + 1) * P], tp)
                st["kbatch"], st["xT16"] = kbatch, xT16

                wcols = gb * C
                kcols = gb * K
                # embed: kemb = (keys & ~0xFF) | c   (one fused DVE op)
                kemb = wk.tile([P, BTC], F32, tag="bt", bufs=2)
                nc.vector.scalar_tensor_tensor(
                    kemb.bitcast(I32)[:, 0:wcols], kbatch.bitcast(I32)[:, 0:wcols],
                    sc_m256[:, 0:1], iotaB[:, 0:wcols], ALU.bitwise_and,
                    ALU.bitwise_or,
                )
                # exact-key low plane extract EARLY (frees kbatch for the next
                # group's ACT refill during the network)
                if not no_cleanup:
                    k16 = kbatch.bitcast(I16)
                    lo_pl = wk.tile([P, BTC], I16, tag="lo_pl", bufs=3)
                    nc.vector.tensor_copy(lo_pl[:, 0:wcols], k16[:, 0 : 2 * wcols : 2])
                    st["lo_pl"] = lo_pl
                if stage == "no_sort":
                    # timing probe: skip the network; take segment prefixes
                    # (embedded idx 0..127, distinct -> valid gather indices)
                    top = wk.tile([P, batch * K], F32, tag="btop", bufs=2)
                    kv = kemb.rearrange("p (n x) -> p n x", n=batch)
                    tb = top.rearrange("p (n x) -> p n x", n=batch)
                    nc.vector.tensor_copy(tb, kv[:, :, 0:K])
                else:
                    top = bitonic_topk(nc, wk, kemb, batch)
                st["top"] = top
                # exact-key base: contiguous DVE snapshot (no cross-engine
                # stall) BEFORE the in-place idx extract
                if not no_cleanup:
                    g32 = wk.tile([P, BTK], F32, tag="g32", bufs=4)
                    nc.vector.tensor_copy(g32[:, 0:kcols], top[:, 0:kcols])
                    st["g32"] = g32
                # idx extract + local segment offset, IN PLACE into top
                tv = top.bitcast(I32)
                nc.vector.scalar_tensor_tensor(
                    tv[:, 0:kcols], tv[:, 0:kcols],
                    sc_ff[:, 0:1], offsB[:, 0:kcols], ALU.bitwise_and,
                    ALU.bitwise_or,  # offsets are 256-multiples: or == add
                )
                # contiguous i16 scatter indices
                idxu = wk.tile([P, BTK], I16, tag="idxu", bufs=2)
                nc.vector.tensor_copy(idxu[:, 0:kcols], tv[:, 0:kcols])
                st["idxu"] = idxu
                return st

            def stB(st):
                """Pool: rank-inversion + gathers.  rankp1 values are
                GLOBAL slot+1 (0 = unselected); gathers write through a
                one-slot pad so no -1 adjustment op is needed (duplicate
                writes of unselected channels land in pad slot 0)."""
                gb = len(st["ts"])
                wcols = gb * C
                kcols = gb * K
                rankp1 = wk.tile([P, BTC], I16, tag="rankp1", bufs=1)
                for h in range(2):
                    t0, t1 = h * half, (h + 1) * half
                    nc.gpsimd.local_scatter(
                        rankp1[:, t0 * C : t1 * C], iota1B[:, t0 * K : t1 * K],
                        st["idxu"][:, t0 * K : t1 * K],
                        channels=P, num_elems=(t1 - t0) * C,
                        num_idxs=(t1 - t0) * K,
                    )
                xg = wk.tile([P, BTK + 2], F16, tag="xg", bufs=3)
                nc.gpsimd.local_scatter(
                    xg, st["xT16"][:, 0:wcols], rankp1[:, 0:wcols],
                    channels=P, num_elems=BTK + 2, num_idxs=wcols,
                )
                st["xg"] = xg
                if no_cleanup:
                    return
                glo = wk.tile([P, BTK + 2], I16, tag="glo", bufs=3)
                nc.gpsimd.local_scatter(
                    glo, st["lo_pl"][:, 0:wcols], rankp1[:, 0:wcols],
                    channels=P, num_elems=BTK + 2, num_idxs=wcols,
                )
                st["glo"] = glo

            def stC(st):
                """DVE: exact-key low-plane patch + product + cleanup L1."""
                gb = len(st["ts"])
                kcols = gb * K
                xgs = st["xg"][:, 1 : 1 + kcols]          # undo +1 pad shift
                if no_cleanup:
                    prod = wk.tile([P, BTK], F32, tag="prodA", bufs=2)
                    nc.vector.tensor_tensor(prod[:, 0:kcols], xgs,
                                            st["top"][:, 0:kcols], ALU.mult)
                    st["prod2"] = prod
                    return
                # g32 low i16 plane := gathered exact plane (bits[15:8] match)
                g32 = st["g32"]
                g16v = g32.bitcast(I16)
                nc.vector.tensor_copy(g16v[:, 0 : 2 * kcols : 2],
                                      st["glo"][:, 1 : 1 + kcols])
                prod = wk.tile([P, BTK], F32, tag="prodA", bufs=2)
                nc.vector.tensor_tensor(prod[:, 0:kcols], xgs,
                                        g32[:, 0:kcols], ALU.mult)
                # cleanup L1: pairs (2q, 2q+1)
                ge = g32[:, 0 : kcols : 2]
                go = g32[:, 1 : kcols : 2]
                m1 = wk.tile([P, BTK // 2], U8, tag="m1", bufs=2)
                nc.vector.tensor_tensor(m1[:, 0 : kcols // 2], ge, go, ALU.is_lt)
                g2 = wk.tile([P, BTK], F32, tag="g2", bufs=2)
                nc.vector.tensor_tensor(g2[:, 0 : kcols : 2], ge, go, ALU.max)
                nc.vector.tensor_tensor(g2[:, 1 : kcols : 2], ge, go, ALU.min)
                p1 = wk.tile([P, BTK], F32, tag="p1", bufs=2)
                nc.scalar.copy(p1[:, 0:kcols], prod[:, 0:kcols])
                nc.vector.copy_predicated(p1[:, 0 : kcols : 2], m1[:, 0 : kcols // 2],
                                          prod[:, 1 : kcols : 2])
                nc.vector.copy_predicated(p1[:, 1 : kcols : 2], m1[:, 0 : kcols // 2],
                                          prod[:, 0 : kcols : 2])
                st["g2"], st["p1"] = g2, p1

            def stD(st):
                """DVE cleanup L2; per-tile transpose + store."""
                gb = len(st["ts"])
                kcols = gb * K
                prod2 = wk.tile([P, BTK], F32, tag="prod2", bufs=2)
                if no_cleanup:
                    prod2 = st["prod2"]
                else:
                    g2, p1 = st["g2"], st["p1"]
                    # L2 pairs (2q+1, 2q+2) within each 128-segment
                    g2B = g2.rearrange("p (n x) -> p n x", x=K)[:, 0:gb]
                    p1B = p1.rearrange("p (n x) -> p n x", x=K)[:, 0:gb]
                    p2B = prod2.rearrange("p (n x) -> p n x", x=K)[:, 0:gb]
                    g2e = g2B[:, :, 1 : K - 1 : 2]
                    g2o = g2B[:, :, 2 : K - 1 : 2]
                    m2 = wk.tile([P, BTK // 2], U8, tag="m2", bufs=2)
                    m2B = m2.rearrange("p (n x) -> p n x", x=K // 2)[:, 0:gb, 0 : K // 2 - 1]
                    nc.vector.tensor_tensor(m2B, g2e, g2o, ALU.is_lt)
                    nc.scalar.copy(prod2[:, 0:kcols], p1[:, 0:kcols])
                    nc.vector.copy_predicated(p2B[:, :, 1 : K - 1 : 2], m2B,
                                              p1B[:, :, 2 : K - 1 : 2])
                    nc.vector.copy_predicated(p2B[:, :, 2 : K - 1 : 2], m2B,
                                              p1B[:, :, 1 : K - 1 : 2])
                for i, t in enumerate(st["ts"]):
                    b, j = divmod(t, COLS)
                    col = j * P
                    op = pp.tile([P, P], F32, tag="otr")
                    nc.tensor.transpose(op, prod2[:, i * K : (i + 1) * K], ident)
                    osb = wk.tile([P, P], F32, tag="osb", bufs=4)
                    nc.scalar.copy(osb, op)
                    nc.sync.dma_start(out=out_ext[b, :, col : col + P], in_=osb)

            def loop_body():
                ring = {}
                ng = len(groups)
                for i in range(ng + 4):
                    if i < ng:
                        ring[i] = stA(groups[i])
                        if stage == "stA":
                            st = ring.pop(i)
                            nc.sync.dma_start(out=out_ext[0, :, 0:P],
                                              in_=st["top"][:, 0:P])
                            continue
                    if 1 <= i and i - 1 in ring:
                        stB(ring[i - 1])
                    if 3 <= i and i - 3 in ring:
                        stC(ring[i - 3])
                    if 4 <= i and i - 4 in ring:
                        stD(ring.pop(i - 4))

            if repeat == 1:
                loop_body()
            else:
                with tc.For_i(0, repeat, 1):
                    loop_body()

    return nc


def _run(inputs, trace: bool = False):
    from concourse.bass_utils import run_bass_kernel_spmd

    x = np.ascontiguousarray(inputs["x"], dtype=np.float32).reshape(B, C, HWP)
    w1 = np.ascontiguousarray(inputs["w1"], dtype=np.float32)
    b1 = np.ascontiguousarray(inputs["b1"], dtype=np.float32).reshape(MED, 1)
    w2 = np.ascontiguousarray(inputs["w2"], dtype=np.float32)
    b2 = np.ascontiguousarray(inputs["b2"], dtype=np.float32).reshape(1, C)
    assert int(inputs.get("out_c", K)) == K

    nc = build()
    nc.finalize()
    core_ids = list(range(NCORES))
    in_maps = [
        {
            "x": np.ascontiguousarray(x[i * BPC : (i + 1) * BPC]),
            "w1": w1, "b1": b1, "w2": w2, "b2": b2,
        }
        for i in core_ids
    ]
    res = None
    for attempt in range(3):
        try:
            res = run_bass_kernel_spmd(nc, in_maps, core_ids, trace=trace)
            break
        except Exception:
            if attempt == 2:
                raise
    out = np.concatenate([r["out"] for r in res.results], axis=0)
    return out.reshape(B, K, H, W), res


def kernel(**inputs) -> np.ndarray:
    out, _ = _run(inputs, trace=False)
    return out


if __name__ == "__main__":
    nc = build(n_tiles=10)
    print("build ok:", nc)
